# revision 8
# baseline (speedup 1.0000x reference)
"""Trainium2 Bass kernel for nn_CNNBlock (proj_in -> scatter -> 2x dilated
conv+syncBN+relu+residual -> gather -> proj_out -> residual -> LayerNorm).

Data-parallel over the batch on 8 NeuronCores; BN batch stats synchronized
with a tiny AllGather per conv layer.

v2: interleaved proj_in/conv emission (PE never drains between stages),
batched 5KB-line DMAs (host p-major layouts), bf16 x/out I/O, residual add
on the TensorE via identity-matmul into PSUM, LN stats straight from PSUM
with multi-chunk bn_stats, ping-pong grid buffers (2 per image, not 3).

Self-contained: hardcodes shapes from the problem spec.
"""
import numpy as np

B = 128          # batch
NCORES = 8
BL = B // NCORES  # 16 images per core
S = 624          # stabilizers = 24*26 grid cells, row-major
H = 256          # hidden
C = 128          # conv channels
G = 26           # grid size
CH = 338         # conv output chunk = 13 rows * 26 cols
SL = BL * S      # 9984 rows per core
EPS = 1e-5
N_BN = B * G * G  # BN stat count per channel (full batch)
PAD = 640        # padded tokens per image (5*128)
# Guarded grid layout: each of the 26 grid rows is 28 wide (cols 26,27 are
# zero guards) plus a 2-element leading zero guard, so every conv tap reads a
# 26-wide window and out-of-row accesses land on zeros.
GW = 28
GOFF = 2
GBUF = GOFF + GW * G + 2  # 732

# wsb32 column map (f32 packed constants)
WO_C = 0          # [C, 256]  W_out
BV_C = 256        # [C, 8]    b_in, bn_g0, bn_b0, bn_g1, bn_b1
WT0_C = 264       # [C, 9*128] conv0 taps
WT1_C = 1416      # [C, 9*128] conv1 taps
W32_N = 2568
# wsb16 column map (bf16 packed constants)
WI_C = 0          # [C, 256]  W_in two k-halves
ID_C = 256        # [C, 128]  identity
W16_N = 384

_CACHE = {}


def _build(ln_affine):
    import concourse.bacc as bacc
    import concourse.tile as tile
    from concourse import mybir

    F32 = mybir.dt.float32
    F32R = mybir.dt.float32r
    BF16 = mybir.dt.bfloat16
    AF = mybir.ActivationFunctionType
    ALU = mybir.AluOpType
    AX = mybir.AxisListType

    nc = bacc.Bacc("TRN2", target_bir_lowering=False, debug=False,
                   enable_asserts=True, num_devices=NCORES)

    xt = nc.dram_tensor("xt", [2 * C, SL], BF16, kind="ExternalInput").ap()
    xrp = nc.dram_tensor("xrp", [C, BL * 5 * H], BF16,
                         kind="ExternalInput").ap()
    w32 = nc.dram_tensor("w32", [C, W32_N], F32, kind="ExternalInput").ap()
    w16 = nc.dram_tensor("w16", [C, W16_N], BF16, kind="ExternalInput").ap()
    if ln_affine:
        lng = nc.dram_tensor("lng", [C, H], F32, kind="ExternalInput").ap()
        lnb = nc.dram_tensor("lnb", [C, H], F32, kind="ExternalInput").ap()
    out = nc.dram_tensor("out", [C, BL * 5 * H], BF16,
                         kind="ExternalOutput").ap()

    with tile.TileContext(nc) as tc:
        with (
            tc.tile_pool(name="const", bufs=1) as const,
            tc.tile_pool(name="grids", bufs=1) as grids,
            tc.tile_pool(name="xg", bufs=3) as xgp,
            tc.tile_pool(name="xr", bufs=6) as xrpool,
            tc.tile_pool(name="osb", bufs=3) as osbp,
            tc.tile_pool(name="tmp", bufs=3) as tmpp,
            tc.tile_pool(name="gat", bufs=4) as gatp,
            tc.tile_pool(name="work", bufs=2) as work,
            tc.tile_pool(name="stats", bufs=1) as stats,
            tc.tile_pool(name="psum", bufs=8, space="PSUM") as psum,
            tc.tile_pool(name="dramp", bufs=4, space="DRAM") as dramp,
        ):
            # ---- packed constants: one DMA each ----
            wsb32 = const.tile([C, W32_N], F32R, name="wsb32")
            nc.scalar.dma_start(wsb32[:], w32[:, :].bitcast(F32R))
            wsb16 = const.tile([C, W16_N], BF16, name="wsb16")
            nc.scalar.dma_start(wsb16[:], w16[:, :])
            wo = wsb32[:, WO_C:WO_C + 256]
            bv = wsb32[:, BV_C:BV_C + 8].bitcast(F32)
            wt0 = [wsb32[:, WT0_C + 128 * t:WT0_C + 128 * (t + 1)]
                   for t in range(9)]
            wt1 = [wsb32[:, WT1_C + 128 * t:WT1_C + 128 * (t + 1)]
                   for t in range(9)]
            wi = [wsb16[:, WI_C + 128 * k:WI_C + 128 * (k + 1)]
                  for k in range(2)]
            ident = wsb16[:, ID_C:ID_C + 128]
            if ln_affine:
                lng_t = const.tile([C, H], F32, name="lng_t")
                nc.scalar.dma_start(lng_t[:], lng[:, :])
                lnb_t = const.tile([C, H], F32, name="lnb_t")
                nc.scalar.dma_start(lnb_t[:], lnb[:, :])

            zf = const.tile([C, 64], F32, name="zf")
            nc.vector.memset(zf[:], 0.0)
            zr = const.tile([C, 64], F32R, name="zr")
            nc.vector.tensor_copy(zr[:], zf[:])

            # Startup barrier: a tiny AllGather issued first absorbs the
            # cross-core launch skew while the front of the kernel runs, so
            # the first real sync-BN AllGather doesn't pay it.
            bar_in = dramp.tile([C, 2], F32, name="bar_in")
            nc.gpsimd.dma_start(bar_in[:], zf[:, 0:2])
            bar_out = dramp.tile([NCORES * C, 2], F32, name="bar_out")
            nc.gpsimd.collective_compute(
                "AllGather", ALU.bypass,
                replica_groups=[list(range(NCORES))],
                ins=[bar_in.opt()], outs=[bar_out.opt()])
            bar_sb = const.tile([C, 2], F32, name="bar_sb")
            nc.gpsimd.dma_start(bar_sb[:], bar_out[0:C, :])
            # eps tiles tied to the barrier so BN math waits for it
            eps_t = const.tile([C, 1], F32, name="eps_t")
            nc.vector.tensor_scalar(eps_t[:], bar_sb[:, 0:1], 0.0, EPS,
                                    ALU.mult, ALU.add)
            eps256_t = const.tile([C, 1], F32, name="eps256_t")
            nc.vector.tensor_scalar(eps256_t[:], bar_sb[:, 0:1], 0.0,
                                    float(H) * EPS, ALU.mult, ALU.add)

            # ---- persistent per-image ping-pong grids ----
            ga = [grids.tile([C, GBUF], F32R, name=f"ga{i}") for i in range(BL)]
            gb = [grids.tile([C, GBUF], F32R, name=f"gb{i}") for i in range(BL)]

            def gview(t):
                return t[:, GOFF:GOFF + G * GW].rearrange(
                    "p (r c) -> p r c", r=G, c=GW)

            for i in range(BL):
                for t, empty_rows in ((ga[i], True), (gb[i], False)):
                    nc.vector.tensor_copy(t[:, 0:GOFF], zr[:, 0:GOFF])
                    nc.vector.tensor_copy(
                        gview(t)[:, :, G:GW],
                        zr[:, 0:2 * G].rearrange("p (r c) -> p r c", r=G, c=2))
                    if empty_rows:
                        # grid rows 24-25 hold no stabilizers -> zeros
                        nc.vector.tensor_copy(
                            t[:, GOFF + 24 * GW:GOFF + 26 * GW],
                            zr[:, 0:2 * GW])

            sqs0 = stats.tile([C, 192], F32, name="sqs0")
            sqs1 = stats.tile([C, 192], F32, name="sqs1")
            zst6 = stats.tile([C, 480], F32, name="zst6")
            nc.vector.memset(zst6[:], 1.0)

            # ================= conv group (one image pair) =================
            def conv_group(grp, src, wt, dil, sqs, dst):
                psc = [psum.tile([C, 512], F32, tag="ps",
                                 name=f"pc{dil}_{grp}_{m}") for m in range(4)]
                for t9 in range(9):
                    di = (t9 // 3 - 1) * dil
                    dj = (t9 % 3 - 1) * dil
                    for m in range(4):
                        img = grp * 2 + m // 2
                        q = m % 2
                        r_lo = max(13 * q, -di)
                        r_hi = min(13 * q + 13, G - di)
                        nr = r_hi - r_lo
                        base = GOFF + (r_lo + di) * GW + dj
                        rhs = src[img][:, base:base + nr * GW].rearrange(
                            "p (r c) -> p r c", r=nr, c=GW)[:, :, 0:G]
                        oap = psc[m][:, (r_lo - 13 * q) * G:
                                      (r_hi - 13 * q) * G]
                        nc.tensor.matmul(oap, wt[t9], rhs,
                                         start=(t9 == 0), stop=(t9 == 8))
                for m in range(4):
                    img = grp * 2 + m // 2
                    q = m % 2
                    cid = img * 2 + q
                    dgv = gview(dst[img])[:, 13 * q:13 * q + 13, 0:G]
                    nc.vector.tensor_copy(
                        dgv, psc[m][:, 0:CH].rearrange("p (r c) -> p r c",
                                                       r=13, c=G))
                    nc.vector.bn_stats(sqs[:, 6 * cid:6 * cid + 6],
                                       psc[m][:, 0:CH])

            # ============ sync-BN stats: AllGather + combine ============
            def bn_sync(sqs, gcol, bcol, tag):
                # sqs = 32 chunks x (cnt_e, mean_e, M2_e, cnt_o, mean_o, M2_o)
                # = 64 half-chunks x (cnt, mean, M2), each over 169 elements.
                v = sqs[:].rearrange("p (k t) -> p t k", k=64, t=3)
                Sm = stats.tile([C, 4], F32, name=f"Sm{tag}")
                nc.vector.tensor_reduce(Sm[:, 0:1], v[:, 1:2, :], AX.X, ALU.add)
                nc.vector.tensor_reduce(Sm[:, 1:2], v[:, 2:3, :], AX.X, ALU.add)
                sqm = stats.tile([C, 64], F32, name=f"sqm{tag}")
                nc.scalar.activation(
                    sqm[:].rearrange("p (a k) -> p a k", a=1),
                    v[:, 1:2, :], AF.Square)
                nc.vector.tensor_reduce(Sm[:, 2:3], sqm[:], AX.X, ALU.add)
                Sl = stats.tile([C, 2], F32, name=f"Sl{tag}")
                nc.vector.tensor_scalar(Sl[:, 0:1], Sm[:, 0:1],
                                        float(CH // 2), None, ALU.mult)
                # sumsq = sum(M2) + 169 * sum(mean^2)
                nc.vector.scalar_tensor_tensor(
                    Sl[:, 1:2], Sm[:, 2:3], float(CH // 2), Sm[:, 1:2],
                    ALU.mult, ALU.add)
                agin = dramp.tile([C, 2], F32, name=f"agin{tag}")
                nc.gpsimd.dma_start(agin[:], Sl[:])
                agout = dramp.tile([NCORES * C, 2], F32, name=f"agout{tag}")
                nc.gpsimd.collective_compute(
                    "AllGather", ALU.bypass,
                    replica_groups=[list(range(NCORES))],
                    ins=[agin.opt()], outs=[agout.opt()])
                gath = stats.tile([C, 16], F32, name=f"gath{tag}")
                nc.gpsimd.dma_start(
                    gath[:].rearrange("p (r k) -> p r k", r=NCORES),
                    agout[:].rearrange("(r p) k -> p r k", r=NCORES, p=C))
                Sg = stats.tile([C, 2], F32, name=f"Sg{tag}")
                nc.vector.tensor_reduce(
                    Sg[:], gath[:].rearrange("p (r k) -> p k r", r=NCORES),
                    AX.X, ALU.add)
                mv = stats.tile([C, 2], F32, name=f"mv{tag}")
                nc.vector.tensor_scalar(mv[:], Sg[:], 1.0 / N_BN, None,
                                        ALU.mult)
                m2 = stats.tile([C, 1], F32, name=f"m2{tag}")
                nc.vector.tensor_tensor(m2[:], mv[:, 0:1], mv[:, 0:1],
                                        ALU.mult)
                var = stats.tile([C, 1], F32, name=f"var{tag}")
                nc.vector.tensor_tensor(var[:], mv[:, 1:2], m2[:],
                                        ALU.subtract)
                std = stats.tile([C, 1], F32, name=f"std{tag}")
                nc.scalar.activation(std[:], var[:], AF.Sqrt,
                                     bias=eps_t[:, 0:1])
                rstd = stats.tile([C, 1], F32, name=f"rstd{tag}")
                nc.vector.reciprocal(rstd[:], std[:])
                a = stats.tile([C, 1], F32, name=f"a{tag}")
                nc.vector.tensor_tensor(a[:], gcol, rstd[:], ALU.mult)
                t1 = stats.tile([C, 1], F32, name=f"t1{tag}")
                nc.vector.tensor_tensor(t1[:], mv[:, 0:1], a[:], ALU.mult)
                sh = stats.tile([C, 1], F32, name=f"sh{tag}")
                nc.vector.tensor_tensor(sh[:], bcol, t1[:], ALU.subtract)
                return a, sh

            # ================= Phase A: proj_in || conv0 =================
            xtv = xt.rearrange("(k p) s -> p k s", k=2)
            for j in range(BL // 2):
                xg = xgp.tile([C, 2496], BF16, name="xg", tag="xg")
                nc.sync.dma_start(
                    xg[:].rearrange("p (k s) -> p k s", k=2),
                    xtv[:, :, 1248 * j:1248 * (j + 1)])
                for e in range(2):
                    i = 2 * j + e
                    for q in range(2):
                        pa = psum.tile([C, 512], F32, tag="ps",
                                       name=f"pa{i}_{q}")
                        o = 624 * e + 312 * q
                        nc.tensor.matmul(pa[:, 0:312], wi[0],
                                         xg[:, o:o + 312],
                                         start=True, stop=False)
                        nc.tensor.matmul(pa[:, 0:312], wi[1],
                                         xg[:, 1248 + o:1248 + o + 312],
                                         start=False, stop=True)
                        dst = gview(ga[i])[:, 12 * q:12 * q + 12, 0:G]
                        nc.scalar.activation(
                            dst, pa[:, 0:312].rearrange("p (r c) -> p r c",
                                                        r=12, c=G),
                            AF.Identity, bias=bv[:, 0:1], scale=1.0)
                if j >= 1:
                    conv_group(j - 1, ga, wt0, 1, sqs0, gb)
            conv_group(BL // 2 - 1, ga, wt0, 1, sqs0, gb)

            a0, sh0 = bn_sync(sqs0, bv[:, 1:2], bv[:, 2:3], "0")

            # ============ Phase B: bn_apply0 || conv1 (ping-pong) ========
            def apply0(i):
                tmp = tmpp.tile([C, 676], F32R, name="t0", tag="tmp")
                tv = tmp[:].rearrange("p (r c) -> p r c", r=G, c=G)
                nc.scalar.activation(tv, gview(gb[i])[:, :, 0:G],
                                     AF.Relu, bias=sh0[:, 0:1],
                                     scale=a0[:, 0:1])
                # gb <- ga + relu(a*gb+sh): gb becomes the conv1 input
                nc.gpsimd.tensor_tensor(gview(gb[i])[:, :, 0:G],
                                        gview(ga[i])[:, :, 0:G], tv, ALU.add)

            for j in range(BL // 2):
                apply0(2 * j)
                apply0(2 * j + 1)
                if j >= 1:
                    conv_group(j - 1, gb, wt1, 2, sqs1, ga)
            conv_group(BL // 2 - 1, gb, wt1, 2, sqs1, ga)

            a1, sh1 = bn_sync(sqs1, bv[:, 3:4], bv[:, 4:5], "1")

            # ==== Phase C: apply1+gather || proj_out+residual+LN+store ====
            zst6v = zst6[:].rearrange("p (g s) -> p g s", g=80, s=6)
            for j in range(BL // 2):
                gats = []
                for e in range(2):
                    i = 2 * j + e
                    xr = xrpool.tile([C, 1280], BF16, name="xr", tag="xr")
                    nc.sync.dma_start(xr[:], xrp[:, 1280 * i:1280 * (i + 1)])
                    tmp = tmpp.tile([C, 676], F32R, name="t1", tag="tmp")
                    t24 = tmp[:, 0:S].rearrange("p (r c) -> p r c", r=24, c=G)
                    nc.scalar.activation(t24, gview(ga[i])[:, 0:24, 0:G],
                                         AF.Relu, bias=sh1[:, 0:1],
                                         scale=a1[:, 0:1])
                    gat = gatp.tile([C, PAD], F32R, name="gat", tag="gat")
                    nc.vector.tensor_copy(gat[:, S:PAD], zr[:, 0:PAD - S])
                    nc.gpsimd.tensor_tensor(
                        gat[:, 0:S].rearrange("p (r c) -> p r c", r=24, c=G),
                        gview(gb[i])[:, 0:24, 0:G], t24, ALU.add)
                    gats.append((gat, xr))
                # proj_out matmuls + residual via identity-matmul, z in PSUM
                zts = [[], []]
                z4 = psum.tile([C, 512], F32, tag="ps", name=f"z4_{j}")
                for e in range(2):
                    gat, xr = gats[e]
                    xrv = xr[:].rearrange("p (t h) -> p t h", t=5)
                    for t in range(5):
                        sz = 128 if t < 4 else 112
                        if t == 4:
                            zt, seg = z4, 256 * e
                        else:
                            if t % 2 == 0:
                                zt = psum.tile([C, 512], F32, tag="ps",
                                               name=f"z{j}_{e}_{t // 2}")
                                zts[e].append(zt)
                            zt, seg = zts[e][t // 2], 256 * (t % 2)
                        nc.tensor.matmul(zt[0:sz, seg:seg + 256],
                                         gat[:, 128 * t:128 * t + sz], wo,
                                         start=True, stop=False)
                        nc.tensor.matmul(zt[0:sz, seg:seg + 256],
                                         ident[0:sz, 0:sz], xrv[0:sz, t, :],
                                         start=False, stop=True)
                    # LN stats straight from PSUM (per-chunk; walrus requires
                    # bn_stats output of exactly 6 elements/partition)
                    i = 2 * j + e
                    for t in range(4):
                        g = 5 * i + t
                        nc.vector.bn_stats(
                            zst6[:, 6 * g:6 * g + 6],
                            zts[e][t // 2][:, 256 * (t % 2):256 * (t % 2) + 256])
                    g = 5 * i + 4
                    nc.vector.bn_stats(zst6[0:112, 6 * g:6 * g + 6],
                                       z4[0:112, 256 * e:256 * e + 256])
                # ---- per-pair mean/var -> apply params ----
                wk = work.tile([C, 112], F32, name="wk", tag="wk")
                V = zst6v[:, 10 * j:10 * j + 10, :]
                me, mo = V[:, :, 1:2], V[:, :, 4:5]
                M2e, M2o = V[:, :, 2:3], V[:, :, 5:6]

                def wkv(c0):
                    return wk[:, c0:c0 + 10].rearrange(
                        "p (g s) -> p g s", g=10)
                # mean2x = me+mo; mu = mean2x/2
                nc.vector.tensor_tensor(wkv(10), me, mo, ALU.add)
                nc.vector.tensor_scalar(wk[:, 0:10], wk[:, 10:20], 0.5, None,
                                        ALU.mult)
                # var*256 = (M2e+M2o) + 64*(me-mo)^2
                nc.gpsimd.tensor_tensor(wkv(20), me, mo, ALU.subtract)
                nc.gpsimd.tensor_tensor(wk[:, 30:40], wk[:, 20:30],
                                        wk[:, 20:30], ALU.mult)
                nc.vector.tensor_tensor(wkv(40), M2e, M2o, ALU.add)
                nc.vector.scalar_tensor_tensor(
                    wk[:, 50:60], wk[:, 30:40], float(H // 4), wk[:, 40:50],
                    ALU.mult, ALU.add)
                # std16 = sqrt(var*256 + 256*eps) = 16*std
                nc.scalar.activation(wk[:, 60:70], wk[:, 50:60], AF.Sqrt,
                                     bias=eps256_t[:, 0:1])
                nc.vector.reciprocal(wk[:, 70:80], wk[:, 60:70])
                # rstd = 16/std16 ; bact = -mu*rstd = mean2x*(1/std16)*(-8)
                nc.vector.tensor_scalar(wk[:, 80:90], wk[:, 70:80], 16.0,
                                        None, ALU.mult)
                nc.vector.tensor_tensor(wk[:, 90:100], wk[:, 10:20],
                                        wk[:, 70:80], ALU.mult)
                nc.vector.tensor_scalar(wk[:, 100:110], wk[:, 90:100], -8.0,
                                        None, ALU.mult)
                # ---- apply + store ----
                for e in range(2):
                    i = 2 * j + e
                    osb = osbp.tile([C, 1280], BF16, name="osb", tag="osb")
                    # rows 112:128 of the last chunk are padding the host
                    # discards; zero them so the store never reads junk
                    nc.vector.memset(osb[:, 1024:1280], 0.0)
                    for t in range(5):
                        sz = 128 if t < 4 else 112
                        zt, seg = (z4, 256 * e) if t == 4 else \
                            (zts[e][t // 2], 256 * (t % 2))
                        mu = wk[0:sz, 5 * e + t:5 * e + t + 1]
                        rs = wk[0:sz, 80 + 5 * e + t:80 + 5 * e + t + 1]
                        ba = wk[0:sz, 100 + 5 * e + t:100 + 5 * e + t + 1]
                        oap = osb[0:sz, 256 * t:256 * (t + 1)]
                        zap = zt[0:sz, seg:seg + 256]
                        if ln_affine:
                            zo = work.tile([C, 256], F32, name="zo", tag="zo",
                                           bufs=4)
                            nc.vector.tensor_scalar(zo[0:sz, :], zap, mu, rs,
                                                    ALU.subtract, ALU.mult)
                            z2 = work.tile([C, 256], F32, name="z2", tag="z2",
                                           bufs=4)
                            nc.vector.tensor_tensor(z2[0:sz, :], zo[0:sz, :],
                                                    lng_t[0:sz, :], ALU.mult)
                            nc.gpsimd.tensor_tensor(oap, z2[0:sz, :],
                                                    lnb_t[0:sz, :], ALU.add)
                        elif t == 3:
                            nc.vector.tensor_scalar(oap, zap, mu, rs,
                                                    ALU.subtract, ALU.mult)
                        else:
                            nc.scalar.activation(oap, zap, AF.Identity,
                                                 bias=ba, scale=rs)
                    nc.sync.dma_start(out[:, 1280 * i:1280 * (i + 1)], osb[:])

    nc.compile()
    return nc


def _get_nc(ln_affine):
    key = ("nc", ln_affine)
    if key not in _CACHE:
        _CACHE[key] = _build(ln_affine)
    return _CACHE[key]


def kernel(x, stab_rows, stab_cols, W_in, b_in,
           conv_w0, conv_b0, bn_g0, bn_b0,
           conv_w1, conv_b1, bn_g1, bn_b1,
           W_out, b_out, ln_g, ln_b, *, _trace=False):
    from concourse.bass_utils import run_bass_kernel_spmd
    import ml_dtypes
    BF = ml_dtypes.bfloat16

    x = np.asarray(x, dtype=np.float32)
    W_in = np.asarray(W_in, dtype=np.float32)
    b_in = np.asarray(b_in, dtype=np.float32)
    conv_w0 = np.asarray(conv_w0, dtype=np.float32)
    conv_w1 = np.asarray(conv_w1, dtype=np.float32)
    bn_g0 = np.asarray(bn_g0, dtype=np.float32)
    bn_b0 = np.asarray(bn_b0, dtype=np.float32)
    bn_g1 = np.asarray(bn_g1, dtype=np.float32)
    bn_b1 = np.asarray(bn_b1, dtype=np.float32)
    W_out = np.asarray(W_out, dtype=np.float32)
    b_out = np.asarray(b_out, dtype=np.float32)
    ln_g = np.asarray(ln_g, dtype=np.float32)
    ln_b = np.asarray(ln_b, dtype=np.float32)
    # conv_b0/conv_b1 are no-ops through training-mode BN (shift-invariant).

    ln_affine = not (np.all(ln_g == 1.0) and np.all(ln_b == 0.0))
    nc = _get_nc(ln_affine)

    w32 = np.zeros((C, W32_N), dtype=np.float32)
    w32[:, WO_C:WO_C + 256] = W_out
    w32[:, BV_C + 0] = b_in
    w32[:, BV_C + 1] = bn_g0
    w32[:, BV_C + 2] = bn_b0
    w32[:, BV_C + 3] = bn_g1
    w32[:, BV_C + 4] = bn_b1
    w32[:, WT0_C:WT0_C + 1152] = (
        conv_w0.transpose(2, 3, 1, 0).reshape(9, C, C)
        .transpose(1, 0, 2).reshape(C, 9 * C))
    w32[:, WT1_C:WT1_C + 1152] = (
        conv_w1.transpose(2, 3, 1, 0).reshape(9, C, C)
        .transpose(1, 0, 2).reshape(C, 9 * C))
    w16 = np.zeros((C, W16_N), dtype=np.float32)
    w16[:, WI_C:WI_C + 256] = (
        W_in.reshape(2, C, C).transpose(1, 0, 2).reshape(C, 2 * C))
    w16[:, ID_C:ID_C + 128] = np.eye(C, dtype=np.float32)
    w16 = w16.astype(BF)

    in_maps = []
    for k in range(NCORES):
        xs = x[k * BL:(k + 1) * BL]
        xpad = np.zeros((BL, PAD, H), dtype=np.float32)
        xpad[:, :S, :] = xs + b_out[None, None, :]
        m = {
            "xt": np.ascontiguousarray(
                xs.transpose(2, 0, 1).reshape(H, SL)).astype(BF),
            "xrp": np.ascontiguousarray(
                xpad.reshape(BL, 5, C, H).transpose(2, 0, 1, 3)
                .reshape(C, BL * 5 * H)).astype(BF),
            "w32": w32,
            "w16": w16,
        }
        if ln_affine:
            m["lng"] = np.ascontiguousarray(
                np.broadcast_to(ln_g[None, :], (C, H)))
            m["lnb"] = np.ascontiguousarray(
                np.broadcast_to(ln_b[None, :], (C, H)))
        in_maps.append(m)

    res = run_bass_kernel_spmd(nc, in_maps, core_ids=list(range(NCORES)),
                               trace=_trace)
    global LAST_EXEC_NS
    LAST_EXEC_NS = res.exec_time_ns
    outs = []
    for k in range(NCORES):
        o = np.asarray(res.results[k]["out"]).astype(np.float32)
        o = o.reshape(C, BL, 5, H).transpose(1, 2, 0, 3)
        outs.append(o.reshape(BL, PAD, H)[:, :S, :])
    return np.concatenate(outs, axis=0).reshape(B, S, H)


LAST_EXEC_NS = None


# revision 12
# speedup vs baseline: 1.0985x; 1.0985x over previous
"""Trainium2 Bass kernel for nn_CNNBlock (proj_in -> scatter -> 2x dilated
conv+syncBN+relu+residual -> gather -> proj_out -> residual -> LayerNorm).

Data-parallel over the batch on 8 NeuronCores; BN batch stats synchronized
with a tiny AllGather per conv layer.

v2: interleaved proj_in/conv emission (PE never drains between stages),
batched 5KB-line DMAs (host p-major layouts), bf16 x/out I/O, residual add
on the TensorE via identity-matmul into PSUM, LN stats straight from PSUM
with multi-chunk bn_stats, ping-pong grid buffers (2 per image, not 3).

Self-contained: hardcodes shapes from the problem spec.
"""
import numpy as np

B = 128          # batch
NCORES = 8
BL = B // NCORES  # 16 images per core
S = 624          # stabilizers = 24*26 grid cells, row-major
H = 256          # hidden
C = 128          # conv channels
G = 26           # grid size
CH = 338         # conv output chunk = 13 rows * 26 cols
SL = BL * S      # 9984 rows per core
EPS = 1e-5
N_BN = B * G * G  # BN stat count per channel (full batch)
PAD = 640        # padded tokens per image (5*128)
# Guarded grid layout: each of the 26 grid rows is 28 wide (cols 26,27 are
# zero guards) plus a 2-element leading zero guard, so every conv tap reads a
# 26-wide window and out-of-row accesses land on zeros.
GW = 28
GOFF = 2
GBUF = GOFF + GW * G + 2  # 732

# wsb32 column map (f32 packed constants)
WO_C = 0          # [C, 256]  W_out
BV_C = 256        # [C, 8]    b_in, bn_g0, bn_b0, bn_g1, bn_b1
WT0_C = 264       # [C, 9*128] conv0 taps
WT1_C = 1416      # [C, 9*128] conv1 taps
W32_N = 2568
# wsb16 column map (bf16 packed constants)
WI_C = 0          # [C, 256]  W_in two k-halves
ID_C = 256        # [C, 128]  identity
W16_N = 384

_CACHE = {}


def _build(ln_affine):
    import concourse.bacc as bacc
    import concourse.tile as tile
    from concourse import mybir

    F32 = mybir.dt.float32
    F32R = mybir.dt.float32r
    BF16 = mybir.dt.bfloat16
    AF = mybir.ActivationFunctionType
    ALU = mybir.AluOpType
    AX = mybir.AxisListType

    nc = bacc.Bacc("TRN2", target_bir_lowering=False, debug=False,
                   enable_asserts=True, num_devices=NCORES)

    xt = nc.dram_tensor("xt", [2 * C, SL], BF16, kind="ExternalInput").ap()
    xrp = nc.dram_tensor("xrp", [C, BL * 5 * H], BF16,
                         kind="ExternalInput").ap()
    w32 = nc.dram_tensor("w32", [C, W32_N], F32, kind="ExternalInput").ap()
    w16 = nc.dram_tensor("w16", [C, W16_N], BF16, kind="ExternalInput").ap()
    if ln_affine:
        lng = nc.dram_tensor("lng", [C, H], F32, kind="ExternalInput").ap()
        lnb = nc.dram_tensor("lnb", [C, H], F32, kind="ExternalInput").ap()
    out = nc.dram_tensor("out", [C, BL * 5 * H], BF16,
                         kind="ExternalOutput").ap()

    with tile.TileContext(nc) as tc:
        with (
            tc.tile_pool(name="const", bufs=1) as const,
            tc.tile_pool(name="grids", bufs=1) as grids,
            tc.tile_pool(name="xg", bufs=3) as xgp,
            tc.tile_pool(name="xr", bufs=6) as xrpool,
            tc.tile_pool(name="osb", bufs=3) as osbp,
            tc.tile_pool(name="tmp", bufs=3) as tmpp,
            tc.tile_pool(name="gat", bufs=4) as gatp,
            tc.tile_pool(name="work", bufs=2) as work,
            tc.tile_pool(name="stats", bufs=1) as stats,
            tc.tile_pool(name="psum", bufs=8, space="PSUM") as psum,
            tc.tile_pool(name="dramp", bufs=4, space="DRAM") as dramp,
        ):
            # ---- packed constants: one DMA each ----
            wsb32 = const.tile([C, W32_N], F32R, name="wsb32")
            nc.scalar.dma_start(wsb32[:], w32[:, :].bitcast(F32R))
            wsb16 = const.tile([C, W16_N], BF16, name="wsb16")
            nc.scalar.dma_start(wsb16[:], w16[:, :])
            wo = wsb32[:, WO_C:WO_C + 256]
            bv = wsb32[:, BV_C:BV_C + 8].bitcast(F32)
            wt0 = [wsb32[:, WT0_C + 128 * t:WT0_C + 128 * (t + 1)]
                   for t in range(9)]
            wt1 = [wsb32[:, WT1_C + 128 * t:WT1_C + 128 * (t + 1)]
                   for t in range(9)]
            wi = [wsb16[:, WI_C + 128 * k:WI_C + 128 * (k + 1)]
                  for k in range(2)]
            ident = wsb16[:, ID_C:ID_C + 128]
            if ln_affine:
                lng_t = const.tile([C, H], F32, name="lng_t")
                nc.scalar.dma_start(lng_t[:], lng[:, :])
                lnb_t = const.tile([C, H], F32, name="lnb_t")
                nc.scalar.dma_start(lnb_t[:], lnb[:, :])

            zf = const.tile([C, 64], F32, name="zf")
            nc.vector.memset(zf[:], 0.0)
            zr = const.tile([C, 64], F32R, name="zr")
            nc.vector.tensor_copy(zr[:], zf[:])

            # Startup barrier: a tiny AllGather issued first absorbs the
            # cross-core launch skew while the front of the kernel runs, so
            # the first real sync-BN AllGather doesn't pay it.
            bar_in = dramp.tile([C, 2], F32, name="bar_in")
            nc.gpsimd.dma_start(bar_in[:], zf[:, 0:2])
            bar_out = dramp.tile([NCORES * C, 2], F32, name="bar_out")
            nc.gpsimd.collective_compute(
                "AllGather", ALU.bypass,
                replica_groups=[list(range(NCORES))],
                ins=[bar_in.opt()], outs=[bar_out.opt()])
            bar_sb = const.tile([C, 2], F32, name="bar_sb")
            nc.gpsimd.dma_start(bar_sb[:], bar_out[0:C, :])
            # eps tiles (tied to the barrier) are emitted just before
            # bn_sync0: emitting them here would park the in-order DVE queue
            # on the barrier and starve the conv drains.
            eps_t = const.tile([C, 1], F32, name="eps_t")
            eps256_t = const.tile([C, 1], F32, name="eps256_t")

            # ---- persistent per-image ping-pong grids ----
            ga = [grids.tile([C, GBUF], F32R, name=f"ga{i}") for i in range(BL)]
            gb = [grids.tile([C, GBUF], F32R, name=f"gb{i}") for i in range(BL)]

            def gview(t):
                return t[:, GOFF:GOFF + G * GW].rearrange(
                    "p (r c) -> p r c", r=G, c=GW)

            for i in range(BL):
                for t, empty_rows in ((ga[i], True), (gb[i], False)):
                    nc.vector.tensor_copy(t[:, 0:GOFF], zr[:, 0:GOFF])
                    nc.vector.tensor_copy(
                        gview(t)[:, :, G:GW],
                        zr[:, 0:2 * G].rearrange("p (r c) -> p r c", r=G, c=2))
                    if empty_rows:
                        # grid rows 24-25 hold no stabilizers -> zeros
                        nc.vector.tensor_copy(
                            t[:, GOFF + 24 * GW:GOFF + 26 * GW],
                            zr[:, 0:2 * GW])

            sqs0 = stats.tile([C, 192], F32, name="sqs0")
            sqs1 = stats.tile([C, 192], F32, name="sqs1")
            zst6 = stats.tile([C, 480], F32, name="zst6")
            nc.vector.memset(zst6[:], 1.0)

            # ================= conv group (one image pair) =================
            def conv_group(grp, src, wt, dil, sqs, dst):
                psc = [psum.tile([C, 512], F32, tag="ps",
                                 name=f"pc{dil}_{grp}_{m}") for m in range(4)]
                for t9 in range(9):
                    di = (t9 // 3 - 1) * dil
                    dj = (t9 % 3 - 1) * dil
                    for m in range(4):
                        img = grp * 2 + m // 2
                        q = m % 2
                        r_lo = max(13 * q, -di)
                        r_hi = min(13 * q + 13, G - di)
                        nr = r_hi - r_lo
                        base = GOFF + (r_lo + di) * GW + dj
                        rhs = src[img][:, base:base + nr * GW].rearrange(
                            "p (r c) -> p r c", r=nr, c=GW)[:, :, 0:G]
                        oap = psc[m][:, (r_lo - 13 * q) * G:
                                      (r_hi - 13 * q) * G]
                        nc.tensor.matmul(oap, wt[t9], rhs,
                                         start=(t9 == 0), stop=(t9 == 8))
                for m in range(4):
                    img = grp * 2 + m // 2
                    q = m % 2
                    cid = img * 2 + q
                    dgv = gview(dst[img])[:, 13 * q:13 * q + 13, 0:G]
                    nc.vector.tensor_copy(
                        dgv, psc[m][:, 0:CH].rearrange("p (r c) -> p r c",
                                                       r=13, c=G))
                    nc.vector.bn_stats(sqs[:, 6 * cid:6 * cid + 6],
                                       psc[m][:, 0:CH])

            # ============ sync-BN stats: AllGather + combine ============
            def bn_sync(sqs, gcol, bcol, tag):
                # sqs = 32 chunks x (cnt_e, mean_e, M2_e, cnt_o, mean_o, M2_o)
                # = 64 half-chunks x (cnt, mean, M2), each over 169 elements.
                v = sqs[:].rearrange("p (k t) -> p t k", k=64, t=3)
                Sm = stats.tile([C, 4], F32, name=f"Sm{tag}")
                nc.vector.tensor_reduce(Sm[:, 0:1], v[:, 1:2, :], AX.X, ALU.add)
                nc.vector.tensor_reduce(Sm[:, 1:2], v[:, 2:3, :], AX.X, ALU.add)
                sqm = stats.tile([C, 64], F32, name=f"sqm{tag}")
                nc.scalar.activation(
                    sqm[:].rearrange("p (a k) -> p a k", a=1),
                    v[:, 1:2, :], AF.Square)
                nc.vector.tensor_reduce(Sm[:, 2:3], sqm[:], AX.X, ALU.add)
                Sl = stats.tile([C, 2], F32, name=f"Sl{tag}")
                nc.vector.tensor_scalar(Sl[:, 0:1], Sm[:, 0:1],
                                        float(CH // 2), None, ALU.mult)
                # sumsq = sum(M2) + 169 * sum(mean^2)
                nc.vector.scalar_tensor_tensor(
                    Sl[:, 1:2], Sm[:, 2:3], float(CH // 2), Sm[:, 1:2],
                    ALU.mult, ALU.add)
                agin = dramp.tile([C, 2], F32, name=f"agin{tag}")
                nc.gpsimd.dma_start(agin[:], Sl[:])
                agout = dramp.tile([NCORES * C, 2], F32, name=f"agout{tag}")
                nc.gpsimd.collective_compute(
                    "AllGather", ALU.bypass,
                    replica_groups=[list(range(NCORES))],
                    ins=[agin.opt()], outs=[agout.opt()])
                gath = stats.tile([C, 16], F32, name=f"gath{tag}")
                nc.gpsimd.dma_start(
                    gath[:].rearrange("p (r k) -> p r k", r=NCORES),
                    agout[:].rearrange("(r p) k -> p r k", r=NCORES, p=C))
                Sg = stats.tile([C, 2], F32, name=f"Sg{tag}")
                nc.vector.tensor_reduce(
                    Sg[:], gath[:].rearrange("p (r k) -> p k r", r=NCORES),
                    AX.X, ALU.add)
                mv = stats.tile([C, 2], F32, name=f"mv{tag}")
                nc.vector.tensor_scalar(mv[:], Sg[:], 1.0 / N_BN, None,
                                        ALU.mult)
                m2 = stats.tile([C, 1], F32, name=f"m2{tag}")
                nc.vector.tensor_tensor(m2[:], mv[:, 0:1], mv[:, 0:1],
                                        ALU.mult)
                var = stats.tile([C, 1], F32, name=f"var{tag}")
                nc.vector.tensor_tensor(var[:], mv[:, 1:2], m2[:],
                                        ALU.subtract)
                std = stats.tile([C, 1], F32, name=f"std{tag}")
                nc.scalar.activation(std[:], var[:], AF.Sqrt,
                                     bias=eps_t[:, 0:1])
                rstd = stats.tile([C, 1], F32, name=f"rstd{tag}")
                nc.vector.reciprocal(rstd[:], std[:])
                a = stats.tile([C, 1], F32, name=f"a{tag}")
                nc.vector.tensor_tensor(a[:], gcol, rstd[:], ALU.mult)
                t1 = stats.tile([C, 1], F32, name=f"t1{tag}")
                nc.vector.tensor_tensor(t1[:], mv[:, 0:1], a[:], ALU.mult)
                sh = stats.tile([C, 1], F32, name=f"sh{tag}")
                nc.vector.tensor_tensor(sh[:], bcol, t1[:], ALU.subtract)
                return a, sh

            # ================= Phase A: proj_in || conv0 =================
            xtv = xt.rearrange("(k p) s -> p k s", k=2)
            for j in range(BL // 2):
                xg = xgp.tile([C, 2496], BF16, name="xg", tag="xg")
                nc.sync.dma_start(
                    xg[:].rearrange("p (k s) -> p k s", k=2),
                    xtv[:, :, 1248 * j:1248 * (j + 1)])
                for e in range(2):
                    i = 2 * j + e
                    for q in range(2):
                        pa = psum.tile([C, 512], F32, tag="ps",
                                       name=f"pa{i}_{q}")
                        o = 624 * e + 312 * q
                        nc.tensor.matmul(pa[:, 0:312], wi[0],
                                         xg[:, o:o + 312],
                                         start=True, stop=False)
                        nc.tensor.matmul(pa[:, 0:312], wi[1],
                                         xg[:, 1248 + o:1248 + o + 312],
                                         start=False, stop=True)
                        dst = gview(ga[i])[:, 12 * q:12 * q + 12, 0:G]
                        nc.scalar.activation(
                            dst, pa[:, 0:312].rearrange("p (r c) -> p r c",
                                                        r=12, c=G),
                            AF.Identity, bias=bv[:, 0:1], scale=1.0)
                if j >= 1:
                    conv_group(j - 1, ga, wt0, 1, sqs0, gb)
            conv_group(BL // 2 - 1, ga, wt0, 1, sqs0, gb)

            nc.vector.tensor_scalar(eps_t[:], bar_sb[:, 0:1], 0.0, EPS,
                                    ALU.mult, ALU.add)
            nc.vector.tensor_scalar(eps256_t[:], bar_sb[:, 0:1], 0.0,
                                    float(H) * EPS, ALU.mult, ALU.add)
            a0, sh0 = bn_sync(sqs0, bv[:, 1:2], bv[:, 2:3], "0")

            # ============ Phase B: bn_apply0 || conv1 (ping-pong) ========
            def apply0(i):
                tmp = tmpp.tile([C, 676], F32R, name="t0", tag="tmp")
                tv = tmp[:].rearrange("p (r c) -> p r c", r=G, c=G)
                nc.scalar.activation(tv, gview(gb[i])[:, :, 0:G],
                                     AF.Relu, bias=sh0[:, 0:1],
                                     scale=a0[:, 0:1])
                # gb <- ga + relu(a*gb+sh): gb becomes the conv1 input
                nc.gpsimd.tensor_tensor(gview(gb[i])[:, :, 0:G],
                                        gview(ga[i])[:, :, 0:G], tv, ALU.add)

            for j in range(BL // 2):
                apply0(2 * j)
                apply0(2 * j + 1)
                if j >= 1:
                    conv_group(j - 1, gb, wt1, 2, sqs1, ga)
            conv_group(BL // 2 - 1, gb, wt1, 2, sqs1, ga)

            a1, sh1 = bn_sync(sqs1, bv[:, 3:4], bv[:, 4:5], "1")

            # ==== Phase C: apply1+gather || proj_out+residual+LN+store ====
            # software-pipelined: pair j+1's gather/matmuls/stats are emitted
            # before pair j's conversion/apply/store so the PE keeps running
            # while the stats->params chain of the previous pair resolves.
            zst6v = zst6[:].rearrange("p (g s) -> p g s", g=80, s=6)

            def pc_front(j):
                gats = []
                for e in range(2):
                    i = 2 * j + e
                    xr = xrpool.tile([C, 1280], BF16, name="xr", tag="xr")
                    nc.sync.dma_start(xr[:], xrp[:, 1280 * i:1280 * (i + 1)])
                    tmp = tmpp.tile([C, 676], F32R, name="t1", tag="tmp")
                    t24 = tmp[:, 0:S].rearrange("p (r c) -> p r c", r=24, c=G)
                    nc.scalar.activation(t24, gview(ga[i])[:, 0:24, 0:G],
                                         AF.Relu, bias=sh1[:, 0:1],
                                         scale=a1[:, 0:1])
                    gat = gatp.tile([C, PAD], F32R, name="gat", tag="gat")
                    nc.vector.tensor_copy(gat[:, S:PAD], zr[:, 0:PAD - S])
                    nc.gpsimd.tensor_tensor(
                        gat[:, 0:S].rearrange("p (r c) -> p r c", r=24, c=G),
                        gview(gb[i])[:, 0:24, 0:G], t24, ALU.add)
                    gats.append((gat, xr))
                # proj_out matmuls + residual via identity-matmul, z in PSUM
                zts = [[], []]
                z4 = psum.tile([C, 512], F32, tag="ps", name=f"z4_{j}")
                for e in range(2):
                    gat, xr = gats[e]
                    xrv = xr[:].rearrange("p (t h) -> p t h", t=5)
                    for t in range(5):
                        sz = 128 if t < 4 else 112
                        if t == 4:
                            zt, seg = z4, 256 * e
                        else:
                            if t % 2 == 0:
                                zt = psum.tile([C, 512], F32, tag="ps",
                                               name=f"z{j}_{e}_{t // 2}")
                                zts[e].append(zt)
                            zt, seg = zts[e][t // 2], 256 * (t % 2)
                        nc.tensor.matmul(zt[0:sz, seg:seg + 256],
                                         gat[:, 128 * t:128 * t + sz], wo,
                                         start=True, stop=False)
                        nc.tensor.matmul(zt[0:sz, seg:seg + 256],
                                         ident[0:sz, 0:sz], xrv[0:sz, t, :],
                                         start=False, stop=True)
                    # LN stats straight from PSUM (per-chunk; walrus requires
                    # bn_stats output of exactly 6 elements/partition)
                    i = 2 * j + e
                    for t in range(4):
                        g = 5 * i + t
                        nc.vector.bn_stats(
                            zst6[:, 6 * g:6 * g + 6],
                            zts[e][t // 2][:, 256 * (t % 2):256 * (t % 2) + 256])
                    g = 5 * i + 4
                    nc.vector.bn_stats(zst6[0:112, 6 * g:6 * g + 6],
                                       z4[0:112, 256 * e:256 * e + 256])
                return zts, z4

            def pc_back(j, zts, z4):
                # ---- per-pair mean/var -> apply params ----
                wk = work.tile([C, 112], F32, name="wk", tag="wk")
                V = zst6v[:, 10 * j:10 * j + 10, :]
                me, mo = V[:, :, 1:2], V[:, :, 4:5]
                M2e, M2o = V[:, :, 2:3], V[:, :, 5:6]

                def wkv(c0):
                    return wk[:, c0:c0 + 10].rearrange(
                        "p (g s) -> p g s", g=10)
                # mean2x = me+mo; mu = mean2x/2
                nc.vector.tensor_tensor(wkv(10), me, mo, ALU.add)
                nc.vector.tensor_scalar(wk[:, 0:10], wk[:, 10:20], 0.5, None,
                                        ALU.mult)
                # var*256 = (M2e+M2o) + 64*(me-mo)^2
                nc.gpsimd.tensor_tensor(wkv(20), me, mo, ALU.subtract)
                nc.gpsimd.tensor_tensor(wk[:, 30:40], wk[:, 20:30],
                                        wk[:, 20:30], ALU.mult)
                nc.vector.tensor_tensor(wkv(40), M2e, M2o, ALU.add)
                nc.vector.scalar_tensor_tensor(
                    wk[:, 50:60], wk[:, 30:40], float(H // 4), wk[:, 40:50],
                    ALU.mult, ALU.add)
                # std16 = sqrt(var*256 + 256*eps) = 16*std
                nc.scalar.activation(wk[:, 60:70], wk[:, 50:60], AF.Sqrt,
                                     bias=eps256_t[:, 0:1])
                nc.vector.reciprocal(wk[:, 70:80], wk[:, 60:70])
                # rstd = 16/std16 ; bact = -mu*rstd = mean2x*(1/std16)*(-8)
                nc.vector.tensor_scalar(wk[:, 80:90], wk[:, 70:80], 16.0,
                                        None, ALU.mult)
                nc.vector.tensor_tensor(wk[:, 90:100], wk[:, 10:20],
                                        wk[:, 70:80], ALU.mult)
                nc.vector.tensor_scalar(wk[:, 100:110], wk[:, 90:100], -8.0,
                                        None, ALU.mult)
                # ---- apply + store ----
                for e in range(2):
                    i = 2 * j + e
                    osb = osbp.tile([C, 1280], BF16, name="osb", tag="osb")
                    # rows 112:128 of the last chunk are padding the host
                    # discards; zero them so the store never reads junk
                    nc.vector.memset(osb[:, 1024:1280], 0.0)
                    for t in range(5):
                        sz = 128 if t < 4 else 112
                        zt, seg = (z4, 256 * e) if t == 4 else \
                            (zts[e][t // 2], 256 * (t % 2))
                        mu = wk[0:sz, 5 * e + t:5 * e + t + 1]
                        rs = wk[0:sz, 80 + 5 * e + t:80 + 5 * e + t + 1]
                        ba = wk[0:sz, 100 + 5 * e + t:100 + 5 * e + t + 1]
                        oap = osb[0:sz, 256 * t:256 * (t + 1)]
                        zap = zt[0:sz, seg:seg + 256]
                        if ln_affine:
                            zo = work.tile([C, 256], F32, name="zo", tag="zo",
                                           bufs=4)
                            nc.vector.tensor_scalar(zo[0:sz, :], zap, mu, rs,
                                                    ALU.subtract, ALU.mult)
                            z2 = work.tile([C, 256], F32, name="z2", tag="z2",
                                           bufs=4)
                            nc.vector.tensor_tensor(z2[0:sz, :], zo[0:sz, :],
                                                    lng_t[0:sz, :], ALU.mult)
                            nc.gpsimd.tensor_tensor(oap, z2[0:sz, :],
                                                    lnb_t[0:sz, :], ALU.add)
                        elif t == 3:
                            nc.vector.tensor_scalar(oap, zap, mu, rs,
                                                    ALU.subtract, ALU.mult)
                        else:
                            nc.scalar.activation(oap, zap, AF.Identity,
                                                 bias=ba, scale=rs)
                    nc.sync.dma_start(out[:, 1280 * i:1280 * (i + 1)], osb[:])

            pending = None
            for j in range(BL // 2):
                zts, z4 = pc_front(j)
                if pending is not None:
                    pc_back(*pending)
                pending = (j, zts, z4)
            pc_back(*pending)

    nc.compile()
    return nc


def _get_nc(ln_affine):
    key = ("nc", ln_affine)
    if key not in _CACHE:
        _CACHE[key] = _build(ln_affine)
    return _CACHE[key]


def kernel(x, stab_rows, stab_cols, W_in, b_in,
           conv_w0, conv_b0, bn_g0, bn_b0,
           conv_w1, conv_b1, bn_g1, bn_b1,
           W_out, b_out, ln_g, ln_b, *, _trace=False):
    from concourse.bass_utils import run_bass_kernel_spmd
    import ml_dtypes
    BF = ml_dtypes.bfloat16

    x = np.asarray(x, dtype=np.float32)
    W_in = np.asarray(W_in, dtype=np.float32)
    b_in = np.asarray(b_in, dtype=np.float32)
    conv_w0 = np.asarray(conv_w0, dtype=np.float32)
    conv_w1 = np.asarray(conv_w1, dtype=np.float32)
    bn_g0 = np.asarray(bn_g0, dtype=np.float32)
    bn_b0 = np.asarray(bn_b0, dtype=np.float32)
    bn_g1 = np.asarray(bn_g1, dtype=np.float32)
    bn_b1 = np.asarray(bn_b1, dtype=np.float32)
    W_out = np.asarray(W_out, dtype=np.float32)
    b_out = np.asarray(b_out, dtype=np.float32)
    ln_g = np.asarray(ln_g, dtype=np.float32)
    ln_b = np.asarray(ln_b, dtype=np.float32)
    # conv_b0/conv_b1 are no-ops through training-mode BN (shift-invariant).

    ln_affine = not (np.all(ln_g == 1.0) and np.all(ln_b == 0.0))
    nc = _get_nc(ln_affine)

    w32 = np.zeros((C, W32_N), dtype=np.float32)
    w32[:, WO_C:WO_C + 256] = W_out
    w32[:, BV_C + 0] = b_in
    w32[:, BV_C + 1] = bn_g0
    w32[:, BV_C + 2] = bn_b0
    w32[:, BV_C + 3] = bn_g1
    w32[:, BV_C + 4] = bn_b1
    w32[:, WT0_C:WT0_C + 1152] = (
        conv_w0.transpose(2, 3, 1, 0).reshape(9, C, C)
        .transpose(1, 0, 2).reshape(C, 9 * C))
    w32[:, WT1_C:WT1_C + 1152] = (
        conv_w1.transpose(2, 3, 1, 0).reshape(9, C, C)
        .transpose(1, 0, 2).reshape(C, 9 * C))
    w16 = np.zeros((C, W16_N), dtype=np.float32)
    w16[:, WI_C:WI_C + 256] = (
        W_in.reshape(2, C, C).transpose(1, 0, 2).reshape(C, 2 * C))
    w16[:, ID_C:ID_C + 128] = np.eye(C, dtype=np.float32)
    w16 = w16.astype(BF)

    in_maps = []
    for k in range(NCORES):
        xs = x[k * BL:(k + 1) * BL]
        xpad = np.zeros((BL, PAD, H), dtype=np.float32)
        xpad[:, :S, :] = xs + b_out[None, None, :]
        m = {
            "xt": np.ascontiguousarray(
                xs.transpose(2, 0, 1).reshape(H, SL)).astype(BF),
            "xrp": np.ascontiguousarray(
                xpad.reshape(BL, 5, C, H).transpose(2, 0, 1, 3)
                .reshape(C, BL * 5 * H)).astype(BF),
            "w32": w32,
            "w16": w16,
        }
        if ln_affine:
            m["lng"] = np.ascontiguousarray(
                np.broadcast_to(ln_g[None, :], (C, H)))
            m["lnb"] = np.ascontiguousarray(
                np.broadcast_to(ln_b[None, :], (C, H)))
        in_maps.append(m)

    res = run_bass_kernel_spmd(nc, in_maps, core_ids=list(range(NCORES)),
                               trace=_trace)
    global LAST_EXEC_NS
    LAST_EXEC_NS = res.exec_time_ns
    outs = []
    for k in range(NCORES):
        o = np.asarray(res.results[k]["out"]).astype(np.float32)
        o = o.reshape(C, BL, 5, H).transpose(1, 2, 0, 3)
        outs.append(o.reshape(BL, PAD, H)[:, :S, :])
    return np.concatenate(outs, axis=0).reshape(B, S, H)


LAST_EXEC_NS = None
